# revision 1
# baseline (speedup 1.0000x reference)
"""Deformable attention for Trainium2 (8 NeuronCores, batch-parallel).

Device (per core, batch b):
  nc_A: offsets/attention projection  oa = query @ [W_off|W_attn] + bias
        (query pre-transposed on host; pure fp32 matmul pipeline)
  nc_B: output projection  out = agg @ W_out + b_out
        (agg pre-transposed + bf16-cast on host; bf16 matmuls, fp32 accum)
Host: softmax over points, bilinear sampling locations, border-clipped
      corner gather from value, attention-weighted reduction (threaded,
      BLAS batched matmuls).

Note: a fully device-side version (DRAM-scratch transposed value + SWDGE
indirect-DMA gather of 128B bilinear column pairs, DVE weighted combine)
validates in CoreSim, but the InstDMACopy dynamic-AP (indirect) lowering
in the deployed neuronx-cc mis-addresses descriptors on hardware
(verified with probe kernels), so the gather stage runs on host here.
"""
import sys

sys.path.insert(0, "/opt/trn_rl_repo")

from concurrent.futures import ThreadPoolExecutor

import numpy as np
import ml_dtypes

import concourse.bass as bass
import concourse.bacc as bacc
import concourse.mybir as mybir
from concourse.tile import TileContext

F32 = mybir.dt.float32
BF16 = mybir.dt.bfloat16
ACTF = mybir.ActivationFunctionType

B, N, C = 8, 8192, 256
Hh, P, D = 8, 4, 32
HH = 128
WW = 128

_CACHE = {}


def _build_proj_nc():
    """oa[n, 0:96] = qT.T @ [W_off | W_attn] + bias (fp32), qT = query.T."""
    nc = bacc.Bacc("TRN2", target_bir_lowering=False, debug=False)
    qT = nc.dram_tensor("qT", [C, N], F32, kind="ExternalInput")
    w_oa = nc.dram_tensor("w_oa", [C, 96], F32, kind="ExternalInput")
    oa = nc.dram_tensor("oa", [N, 96], F32, kind="ExternalOutput")

    CH = 512  # n per outer chunk
    with TileContext(nc) as tc:
        with tc.tile_pool(name="c", bufs=1) as cp, \
             tc.tile_pool(name="m", bufs=3) as mp, \
             tc.tile_pool(name="ps", bufs=6, space="PSUM") as pp:
            woa_t = cp.tile([128, 2, 96], F32, tag="woa")
            nc.sync.dma_start(woa_t[:],
                              w_oa[:].rearrange("(a p) j -> p a j", p=128))

            for ch in range(N // CH):
                qt_t = mp.tile([128, 2, CH], F32, tag="qt")
                nc.sync.dma_start(
                    qt_t[:],
                    qT[:, ch * CH:(ch + 1) * CH]
                    .rearrange("(a p) n -> p a n", p=128))
                o_sb = mp.tile([128, CH // 128, 96], F32, tag="osb")
                for s in range(CH // 128):
                    poa = pp.tile([128, 96], F32, tag="poa")
                    nc.tensor.matmul(poa[:],
                                     qt_t[:, 0, s * 128:(s + 1) * 128],
                                     woa_t[:, 0, :], start=True, stop=False)
                    nc.tensor.matmul(poa[:],
                                     qt_t[:, 1, s * 128:(s + 1) * 128],
                                     woa_t[:, 1, :], start=False, stop=True)
                    nc.scalar.activation(o_sb[:, s], poa[:], ACTF.Copy)
                nc.sync.dma_start(
                    oa[ch * CH:(ch + 1) * CH, :]
                    .rearrange("(s p) j -> p s j", p=128),
                    o_sb[:])
    nc.compile()
    return nc


def _build_out_nc():
    """out = aggT.T @ W_out + b_out (bf16 matmuls, fp32 accumulate)."""
    nc = bacc.Bacc("TRN2", target_bir_lowering=False, debug=False)
    aggT = nc.dram_tensor("aggT", [C, N], BF16, kind="ExternalInput")
    wout = nc.dram_tensor("wout", [C, C], BF16, kind="ExternalInput")
    bias_out = nc.dram_tensor("bias_out", [128, 2], F32, kind="ExternalInput")
    outT = nc.dram_tensor("outT", [C, N], F32, kind="ExternalOutput")

    CH = 512
    with TileContext(nc) as tc:
        with tc.tile_pool(name="c", bufs=1) as cp, \
             tc.tile_pool(name="m", bufs=3) as mp, \
             tc.tile_pool(name="ps", bufs=4, space="PSUM") as pp:
            wout_t = cp.tile([128, 2, C], BF16, tag="wout")
            nc.sync.dma_start(wout_t[:],
                              wout[:].rearrange("(a p) j -> p a j", p=128))
            bout_t = cp.tile([128, 2], F32, tag="bout")
            nc.sync.dma_start(bout_t[:], bias_out[:])

            for ch in range(N // CH):
                at_t = mp.tile([128, 2, CH], BF16, tag="at")
                nc.sync.dma_start(
                    at_t[:],
                    aggT[:, ch * CH:(ch + 1) * CH]
                    .rearrange("(a p) n -> p a n", p=128))
                for mh in range(2):
                    po = pp.tile([128, CH], F32, tag="po")
                    nc.tensor.matmul(po[:],
                                     wout_t[:, 0, mh * 128:(mh + 1) * 128],
                                     at_t[:, 0, :], start=True, stop=False)
                    nc.tensor.matmul(po[:],
                                     wout_t[:, 1, mh * 128:(mh + 1) * 128],
                                     at_t[:, 1, :], start=False, stop=True)
                    o_sb = mp.tile([128, CH], F32, tag="osb")
                    nc.scalar.activation(o_sb[:], po[:], ACTF.Identity,
                                         bias=bout_t[:, mh:mh + 1])
                    nc.sync.dma_start(
                        outT[mh * 128:(mh + 1) * 128,
                             ch * CH:(ch + 1) * CH], o_sb[:])
    nc.compile()
    return nc


def _proj_host(query, W_off, b_off, W_attn, b_attn):
    w_oa = np.concatenate([W_off, W_attn], axis=1).astype(np.float32)
    b_oa = np.concatenate([b_off, b_attn]).astype(np.float32)
    return query.reshape(-1, C) @ w_oa + b_oa


def _sample_host(oa, reference_points, value):
    """Host bilinear sampling + attention-weighted reduce for one batch."""
    offs = oa[:, :64].reshape(N, Hh, P, 2)
    logits = oa[:, 64:96].reshape(N, Hh, P)
    e = np.exp(logits - logits.max(axis=-1, keepdims=True))
    attn = e / e.sum(axis=-1, keepdims=True)            # (N, Hh, P)

    ref = reference_points * 2.0 - 1.0                   # (N, 2)
    x = (ref[:, None, None, 0] + offs[..., 0] + 1.0) * (WW * 0.5) - 0.5
    y = (ref[:, None, None, 1] + offs[..., 1] + 1.0) * (HH * 0.5) - 0.5
    x0 = np.floor(x).astype(np.int64)
    y0 = np.floor(y).astype(np.int64)
    wx = (x - x0).astype(np.float32)
    wy = (y - y0).astype(np.float32)

    val = np.ascontiguousarray(
        value.reshape(Hh, D, HH, WW).transpose(0, 2, 3, 1))  # (Hh, H, W, D)
    valf = val.reshape(Hh * HH * WW, D)

    hbase = (np.arange(Hh) * (HH * WW))[None, :, None]
    agg = np.zeros((N, Hh, D), np.float32)
    for dy, dx, w in ((0, 0, (1 - wx) * (1 - wy)), (0, 1, wx * (1 - wy)),
                      (1, 0, (1 - wx) * wy), (1, 1, wx * wy)):
        ix = x0 + dx
        iy = y0 + dy
        valid = (ix >= 0) & (ix < WW) & (iy >= 0) & (iy < HH)
        idx = hbase + np.clip(iy, 0, HH - 1) * WW + np.clip(ix, 0, WW - 1)
        g = valf[idx]                                 # (N, Hh, P, D)
        cw = (w * valid * attn).astype(np.float32)    # (N, Hh, P)
        # batched matmul (BLAS, releases GIL): (N*Hh,1,P) @ (N*Hh,P,D)
        agg += np.matmul(cw.reshape(N * Hh, 1, P),
                         g.reshape(N * Hh, P, D)).reshape(N, Hh, D)
    return agg.reshape(N, C)


def _run_spmd(nc, in_maps):
    from concourse.bass_utils import run_bass_kernel_spmd
    return run_bass_kernel_spmd(nc, in_maps, core_ids=list(range(len(in_maps))))


_G = {}


def _sample_worker(b):
    return _sample_host(_G["oa"][b], _G["rp"][b], _G["value"][b])


def _sample_all(oa, reference_points, value):
    """Per-batch sampling in threads. (A fork-Pool variant is ~2x faster on
    the gather but JAX's runtime threads make os.fork() deadlock-prone, so
    threads are used for robustness; BLAS matmuls still parallelize.)"""
    _G.update(oa=oa, rp=reference_points, value=value)
    with ThreadPoolExecutor(max_workers=B) as ex:
        aggs = list(ex.map(_sample_worker, range(B)))
    return np.stack(aggs, axis=0)


def kernel(query, reference_points, value, W_off, b_off, W_attn, b_attn,
           W_out, b_out, H=None, W=None):
    query = np.asarray(query, np.float32)
    reference_points = np.asarray(reference_points, np.float32)
    value = np.asarray(value, np.float32)
    W_off = np.asarray(W_off, np.float32)
    b_off = np.asarray(b_off, np.float32)
    W_attn = np.asarray(W_attn, np.float32)
    b_attn = np.asarray(b_attn, np.float32)
    W_out = np.asarray(W_out, np.float32)
    b_out = np.asarray(b_out, np.float32)

    w_oa = np.concatenate([W_off, W_attn], axis=1).astype(np.float32)
    bias_oa = np.concatenate([b_off, b_attn]).astype(np.float32)[None, :]
    wout_bf = W_out.astype(ml_dtypes.bfloat16)
    bout_2 = np.ascontiguousarray(
        b_out.astype(np.float32).reshape(2, 128).T)  # [128, 2] cout halves

    # ---- stage A: projections on device (fp32) ----
    oa = None
    try:
        if "A" not in _CACHE:
            _CACHE["A"] = _build_proj_nc()
        in_maps = [dict(qT=np.ascontiguousarray(query[b].T), w_oa=w_oa)
                   for b in range(B)]
        res = _run_spmd(_CACHE["A"], in_maps)
        oa = np.stack([res.results[b]["oa"] for b in range(B)], axis=0)
        oa = oa + bias_oa
        if not np.isfinite(oa).all():
            oa = None
    except Exception:
        oa = None
    if oa is None:  # fallback
        oa = np.stack([_proj_host(query[b], W_off, b_off, W_attn, b_attn)
                       for b in range(B)], axis=0)

    # ---- stage S: bilinear sampling + weighted reduce (host, forked) ----
    agg = _sample_all(oa, reference_points, value)

    # ---- stage B: output projection on device (bf16 matmul) ----
    out = None
    try:
        if "B" not in _CACHE:
            _CACHE["B"] = _build_out_nc()
        in_maps = [dict(aggT=np.ascontiguousarray(agg[b].T)
                        .astype(ml_dtypes.bfloat16),
                        wout=wout_bf, bias_out=bout_2)
                   for b in range(B)]
        res = _run_spmd(_CACHE["B"], in_maps)
        out = np.stack([np.ascontiguousarray(res.results[b]["outT"].T)
                        for b in range(B)], axis=0)
        if not np.isfinite(out).all():
            out = None
    except Exception:
        out = None
    if out is None:  # fallback
        out = agg @ W_out + b_out

    return out.astype(np.float32)


if __name__ == "__main__":
    _build_proj_nc()
    _build_out_nc()
    print("built ok")



# revision 2
# speedup vs baseline: 4.0173x; 4.0173x over previous
"""Deformable attention on Trainium2 — fully fused device kernel.

One batch per NeuronCore (8 cores). Per core, a single Bass/Tile NEFF:
  T. value (bf16) -> fp32 sampling table vt[(h,y,k), 2 cells x 32 d]
     via PE transposes (256B rows; gather elem = 2 rows = 4 cells).
  A. per 256-query chunk: qT via PE transpose; oa = Woa^T @ qT in coef
     layout [96, n]; softmax-attn via PE partition-sum + DVE reciprocal;
     bilinear positions/weights/int16 gather indices on DVE/Act.
  B. coef -> descriptor layout (SBUF-SBUF DMAs); SWDGE dma_gather of
     4-cell windows from vt; DVE weighted cell-fold; PE matmul reduce
     over (point, y-row); bf16 agg -> DRAM -> xbar DMA transpose.
  C. out = Wout^T @ aggT + b (bf16 matmul), xbar back to [n, c] bf16.

Launches once per call through a cached jitted shard_map (PJRT), so the
warm path is one NEFF execution plus host<->device transfers. Falls back
to a host numpy pipeline (+ device projections) on any device failure.
"""
import sys

sys.path.insert(0, "/opt/trn_rl_repo")

from concurrent.futures import ThreadPoolExecutor

import numpy as np
import ml_dtypes

import concourse.bass as bass
import concourse.bacc as bacc
import concourse.mybir as mybir
from concourse.tile import TileContext
from concourse import library_config

F32 = mybir.dt.float32
F16 = mybir.dt.float16
BF16 = mybir.dt.bfloat16
I32 = mybir.dt.int32
I16 = mybir.dt.int16
ACT = mybir.ActivationFunctionType
ALU = mybir.AluOpType

B, N, C = 8, 8192, 256
Hh, P, D = 8, 4, 32
HH = WW = 128
RPH = HH * (WW // 2)      # 8192 table rows per head (row = 2 cells x 32 d)
GR = 4 * RPH              # rows per 4-head gather group
NC = 256                  # queries per chunk

_CACHE = {}


# ====================== device kernel ======================

def build_nc(NQ=N):
    NCH = NQ // NC
    nc = bacc.Bacc("TRN2", target_bir_lowering=False, debug=False)

    q16 = nc.dram_tensor("q16", [NQ, C], F16, kind="ExternalInput")
    v16 = nc.dram_tensor("v16", [C, HH, WW], BF16, kind="ExternalInput")
    rp = nc.dram_tensor("rp", [NQ, 2], F32, kind="ExternalInput")
    wq = nc.dram_tensor("wq", [C, 96], F32, kind="ExternalInput")
    boa_d = nc.dram_tensor("boa", [96, 1], F32, kind="ExternalInput")
    wout_d = nc.dram_tensor("wout", [C, C], BF16, kind="ExternalInput")
    bout_d = nc.dram_tensor("bout", [128, 2], F32, kind="ExternalInput")
    hb_d = nc.dram_tensor("hb", [32, 1], F32, kind="ExternalInput")
    patt_d = nc.dram_tensor("patt", [32, 8], F32, kind="ExternalInput")
    stat_d = nc.dram_tensor("stat16", [128, 16], F32, kind="ExternalInput")
    id_d = nc.dram_tensor("id128", [128, 128], F32, kind="ExternalInput")
    out16 = nc.dram_tensor("out16", [NQ, C], BF16, kind="ExternalOutput")

    vt = nc.dram_tensor("vt", [2 * GR + 2, 64], F32, kind="Internal")
    agg_d = nc.dram_tensor("agg_d", [NCH, 2, NC, 128], BF16, kind="Internal")

    with TileContext(nc) as tc:
        nc.gpsimd.load_library(library_config.mlp)

        with tc.tile_pool(name="cst", bufs=1) as cp:
            id_t = cp.tile([128, 128], F32, tag="id")
            nc.sync.dma_start(id_t[:], id_d[:])
            wq_t = cp.tile([128, 2, 96], F32, tag="wq")
            nc.sync.dma_start(wq_t[:], wq[:].rearrange("(a p) j -> p a j", p=128))
            boa_t = cp.tile([96, 1], F32, tag="boa")
            nc.sync.dma_start(boa_t[:], boa_d[:])
            patt_t = cp.tile([32, 8], F32, tag="patt")
            nc.sync.dma_start(patt_t[:], patt_d[:])
            hb_t = cp.tile([32, 1], F32, tag="hb")
            nc.sync.dma_start(hb_t[:], hb_d[:])
            stat_t = cp.tile([128, 16], F32, tag="stat")
            nc.sync.dma_start(stat_t[:], stat_d[:])
            wout_t = cp.tile([128, 2, 256], BF16, tag="wout")
            nc.sync.dma_start(wout_t[:], wout_d[:].rearrange("(a p) j -> p a j", p=128))
            bout_t = cp.tile([128, 2], F32, tag="bout")
            nc.sync.dma_start(bout_t[:], bout_d[:])

            # ---------- stage T: value -> fp32 table ----------
            with tc.tile_pool(name="tb", bufs=2) as tbp, \
                 tc.tile_pool(name="tbq", bufs=2, space="PSUM") as tqp:
                zt = tbp.tile([1, 128], F32, tag="zt")
                nc.vector.memset(zt[:], 0.0)
                nc.sync.dma_start(
                    bass.AP(vt, 2 * GR * 64, [(64, 2), (1, 64)]), zt[:])
                for h in range(8):
                    for yb in range(4):
                        vsb = tbp.tile([32, 32, 128], BF16, tag="vs")
                        nc.sync.dma_start(
                            vsb[:],
                            v16[h * 32:(h + 1) * 32, yb * 32:(yb + 1) * 32, :])
                        vf = tbp.tile([32, 32, 128], F32, tag="vf")
                        nc.scalar.activation(vf[:], vsb[:], ACT.Copy)
                        for half in range(2):
                            pt = tqp.tile([128, 16, 32], F32, tag="pt")
                            for yy in range(16):
                                nc.tensor.transpose(
                                    pt[:, yy, :], vf[:, half * 16 + yy, :],
                                    id_t[0:32, 0:32])
                            st = tbp.tile([128, 16, 32], F32, tag="st")
                            nc.scalar.activation(st[:], pt[:], ACT.Copy)
                            y0 = yb * 32 + half * 16
                            dst = bass.AP(
                                vt, h * RPH * 64 + y0 * 4096,
                                [(32, 128), (4096, 16), (1, 32)])
                            nc.sync.dma_start(dst, st[:])

            # ---------- main loop ----------
            with tc.tile_pool(name="m", bufs=2) as mp, \
                 tc.tile_pool(name="cf", bufs=1) as cf, \
                 tc.tile_pool(name="sc", bufs=2) as sc, \
                 tc.tile_pool(name="gp", bufs=1) as gp, \
                 tc.tile_pool(name="fd", bufs=2) as fd, \
                 tc.tile_pool(name="pq", bufs=1, space="PSUM") as pqp, \
                 tc.tile_pool(name="px", bufs=1, space="PSUM") as pxp, \
                 tc.tile_pool(name="pa", bufs=2, space="PSUM") as pap, \
                 tc.tile_pool(name="pu", bufs=2, space="PSUM") as pup:

                def ctile(tag, shape=(32, NC), dtype=F32, pool=None):
                    return (pool or cf).tile(list(shape), dtype, tag=tag,
                                             name=tag)

                for ch in range(NCH):
                    n0 = ch * NC
                    # ---- qT ----
                    qt16 = mp.tile([128, 2, 256], F16, tag="q16")
                    nc.sync.dma_start(
                        qt16[:],
                        q16[n0:n0 + NC, :].rearrange("(a p) c -> p a c", p=128))
                    qf = mp.tile([128, 2, 256], F32, tag="qf")
                    nc.scalar.activation(qf[:], qt16[:], ACT.Copy)
                    pqt = pqp.tile([128, 2, 256], F32, tag="pqt")
                    for nh in range(2):
                        for chh in range(2):
                            nc.tensor.transpose(
                                pqt[:, chh, nh * 128:(nh + 1) * 128],
                                qf[:, nh, chh * 128:(chh + 1) * 128], id_t[:])
                    qsb = mp.tile([128, 2, 256], F32, tag="qsb")
                    nc.scalar.activation(qsb[:], pqt[:], ACT.Copy)

                    # ---- oa (coef layout [96, n]) ----
                    poa = pxp.tile([96, NC], F32, tag="poa")
                    nc.tensor.matmul(poa[:], wq_t[:, 0, :], qsb[:, 0, :],
                                     start=True, stop=False)
                    nc.tensor.matmul(poa[:], wq_t[:, 1, :], qsb[:, 1, :],
                                     start=False, stop=True)
                    oa = ctile("oa", (96, NC))
                    nc.scalar.activation(oa[:], poa[:], ACT.Identity,
                                         bias=boa_t[:, 0:1])
                    offx, offy, lg = oa[0:32, :], oa[32:64, :], oa[64:96, :]

                    # ---- softmax over p ----
                    e = ctile("e")
                    nc.scalar.activation(e[:], lg, ACT.Exp)
                    pse = pxp.tile([8, NC], F32, tag="pse")
                    nc.tensor.matmul(pse[:], patt_t[:], e[:], start=True,
                                     stop=True)
                    rb8 = ctile("rb8", (8, NC))
                    nc.vector.reciprocal(rb8[:], pse[:])
                    rb = ctile("rb")
                    nc.sync.dma_start(
                        rb[:], rb8[:].unsqueeze(1).broadcast_to([8, 4, NC]))
                    attn = ctile("attn")
                    nc.vector.tensor_mul(attn[:], e[:], rb[:])

                    # ---- positions ----
                    rpt = mp.tile([1, NC, 2], F32, tag="rpt")
                    nc.sync.dma_start(rpt[:], rp[n0:n0 + NC, :])
                    rpx1 = mp.tile([1, NC], F32, tag="rpx1")
                    nc.scalar.activation(rpx1[:], rpt[:, :, 0], ACT.Copy,
                                         bias=-0.5, scale=128.0)
                    rpy1 = mp.tile([1, NC], F32, tag="rpy1")
                    nc.scalar.activation(rpy1[:], rpt[:, :, 1], ACT.Copy,
                                         bias=-0.5, scale=128.0)
                    rpx = ctile("rpx")
                    nc.sync.dma_start(
                        rpx[:], rpx1[:].unsqueeze(1).broadcast_to([1, 32, NC]))
                    rpy = ctile("rpy")
                    nc.sync.dma_start(
                        rpy[:], rpy1[:].unsqueeze(1).broadcast_to([1, 32, NC]))
                    x = ctile("x")
                    nc.vector.scalar_tensor_tensor(x[:], offx, 64.0, rpx[:],
                                                   ALU.mult, ALU.add)
                    y = ctile("y")
                    nc.vector.scalar_tensor_tensor(y[:], offy, 64.0, rpy[:],
                                                   ALU.mult, ALU.add)

                    def floorv(v, tag):
                        # floor() robust to trunc- or round-to-nearest casts
                        vi = ctile("fli", dtype=I32, pool=sc)
                        nc.scalar.activation(vi[:], v, ACT.Copy)
                        vf_ = ctile("flf", pool=sc)
                        nc.scalar.activation(vf_[:], vi[:], ACT.Copy)
                        gt_ = ctile("flg", pool=sc)
                        nc.vector.tensor_tensor(gt_[:], vf_[:], v, ALU.is_gt)
                        fl = ctile(tag)
                        nc.vector.tensor_sub(fl[:], vf_[:], gt_[:])
                        return fl

                    x0f = floorv(x[:], "x0f")
                    y0f = floorv(y[:], "y0f")
                    wx = ctile("wx")
                    nc.vector.tensor_sub(wx[:], x[:], x0f[:])
                    wy = ctile("wy")
                    nc.vector.tensor_sub(wy[:], y[:], y0f[:])

                    def in_range(v, lo, hi, tag):
                        a_ = ctile("ira", pool=sc)
                        nc.vector.tensor_single_scalar(a_[:], v, lo, ALU.is_ge)
                        b_ = ctile("irb", pool=sc)
                        nc.vector.tensor_single_scalar(b_[:], v, hi, ALU.is_le)
                        o_ = ctile(tag)
                        nc.vector.tensor_mul(o_[:], a_[:], b_[:])
                        return o_

                    vx0 = in_range(x0f[:], 0.0, 127.0, "vx0")
                    vx1 = in_range(x0f[:], -1.0, 126.0, "vx1")
                    vy0 = in_range(y0f[:], 0.0, 127.0, "vy0")
                    vy1 = in_range(y0f[:], -1.0, 126.0, "vy1")

                    onemwx = ctile("omx", pool=sc)
                    nc.scalar.activation(onemwx[:], wx[:], ACT.Copy,
                                         bias=1.0, scale=-1.0)
                    onemwy = ctile("omy")
                    nc.scalar.activation(onemwy[:], wy[:], ACT.Copy,
                                         bias=1.0, scale=-1.0)
                    wxv0 = ctile("wxv0")
                    nc.vector.tensor_mul(wxv0[:], onemwx[:], vx0[:])
                    wxv1 = ctile("wxv1")
                    nc.vector.tensor_mul(wxv1[:], wx[:], vx1[:])

                    xc = ctile("xc", pool=sc)
                    nc.vector.tensor_scalar(xc[:], x0f[:], 0.0, 126.0,
                                            ALU.max, ALU.min)
                    xh = ctile("xh", pool=sc)
                    nc.scalar.activation(xh[:], xc[:], ACT.Copy, scale=0.5)
                    kxf = floorv(xh[:], "kxf")
                    cellb = ctile("cb", pool=sc)
                    nc.scalar.activation(cellb[:], kxf[:], ACT.Copy, scale=2.0)
                    j0 = ctile("j0")
                    nc.vector.tensor_sub(j0[:], x0f[:], cellb[:])
                    eqs = []
                    for cc in (-1.0, 0.0, 1.0, 2.0, 3.0):
                        eq = ctile(f"eq{int(cc)}")
                        nc.vector.tensor_single_scalar(eq[:], j0[:], cc,
                                                       ALU.is_equal)
                        eqs.append(eq)

                    idx16 = cf.tile([32, 2, NC], I16, tag="idx", name="idx16")
                    wcoef = cf.tile([32, 2, NC, 4], F32, tag="wcf",
                                    name="wcoef")
                    y1f = ctile("y1f", pool=sc)
                    nc.vector.tensor_scalar_add(y1f[:], y0f[:], 1.0)
                    for r, (yrf, vyr, wyr) in enumerate(
                            ((y0f, vy0, onemwy), (y1f, vy1, wy))):
                        ya = ctile("ya", pool=sc)
                        nc.vector.tensor_scalar(ya[:], yrf[:], 0.0, 127.0,
                                                ALU.max, ALU.min)
                        idxf = ctile("ixf", pool=sc)
                        nc.vector.scalar_tensor_tensor(
                            idxf[:], ya[:], 64.0, kxf[:], ALU.mult, ALU.add)
                        idxf2 = ctile("ixg", pool=sc)
                        nc.vector.tensor_scalar_add(idxf2[:], idxf[:],
                                                    hb_t[:, 0:1])
                        nc.scalar.activation(idx16[:, r, :], idxf2[:], ACT.Copy)
                        wyv = ctile("wyv", pool=sc)
                        nc.vector.tensor_mul(wyv[:], wyr[:], vyr[:])
                        base = ctile("bse", pool=sc)
                        nc.vector.tensor_mul(base[:], attn[:], wyv[:])
                        wA = ctile("wA", pool=sc)
                        nc.vector.tensor_mul(wA[:], base[:], wxv0[:])
                        wB = ctile("wB", pool=sc)
                        nc.vector.tensor_mul(wB[:], base[:], wxv1[:])
                        for cc in range(4):
                            t1 = ctile("wt1", pool=sc)
                            nc.vector.tensor_mul(t1[:], wA[:], eqs[cc + 1][:])
                            t2 = ctile("wt2", pool=sc)
                            nc.vector.tensor_mul(t2[:], wB[:], eqs[cc][:])
                            nc.vector.tensor_add(wcoef[:, r, :, cc],
                                                 t1[:], t2[:])

                    # ---- gather + fold + reduce per 4-head group ----
                    aggT = [None, None]
                    for g in range(2):
                        it = mp.tile([128, 512], I16, tag="it")
                        itv = it[0:16, :].rearrange(
                            "p (k y a) -> p y k a", y=2, a=4)
                        for yr in range(2):
                            nc.sync.dma_start(
                                itv[:, yr, :, :],
                                idx16[g * 16:(g + 1) * 16, yr, :])
                        for rep in range(1, 8):
                            nc.sync.dma_start(
                                it[rep * 16:(rep + 1) * 16, :], it[0:16, :])
                        wt = mp.tile([128, 64, 4], F32, tag="wt")
                        wv = wcoef[g * 16:(g + 1) * 16, :, :, :].rearrange(
                            "p y (k a) c -> p y a k c", a=4)
                        for yr in range(2):
                            for a in range(4):
                                nc.sync.dma_start(
                                    wt[yr * 64 + a * 16:
                                       yr * 64 + (a + 1) * 16, :, :],
                                    wv[:, yr, a, :, :])
                        gt = gp.tile([128, 64, 4, 32], F32, tag="gt")
                        src_g = bass.AP(vt, g * GR * 64, [(64, GR), (1, 128)])
                        nc.gpsimd.dma_gather(
                            gt[:].rearrange("p a b c -> p a (b c)"),
                            src_g, it[:], 8192, 8192, 128, elem_step=64)
                        red = None
                        for cc in range(4):
                            t_ = fd.tile([128, 64, 32], F32, tag="fm")
                            nc.vector.tensor_mul(
                                t_[:], gt[:, :, cc, :],
                                wt[:, :, cc].unsqueeze(2)
                                .broadcast_to([128, 64, 32]))
                            if red is None:
                                red = t_
                            else:
                                r_ = fd.tile([128, 64, 32], F32, tag="fr")
                                nc.vector.tensor_add(r_[:], red[:], t_[:])
                                red = r_
                        asb = mp.tile([16, 4, 16, 32], BF16, tag=f"asb{g}")
                        for qq in range(4):
                            pag = pap.tile([16, 512], F32, tag="pag")
                            nc.tensor.matmul(
                                pag[:], stat_t[:],
                                red[:, qq * 16:(qq + 1) * 16, :],
                                start=True, stop=True)
                            nc.scalar.activation(
                                asb[:, qq, :, :],
                                pag[:].rearrange("p (a b) -> p a b", a=16),
                                ACT.Copy)
                        for h2 in range(4):
                            dst = bass.AP(
                                agg_d, ((ch * 2 + g) * NC) * 128 + h2 * 32,
                                [(128, 4), (512, 64), (1, 32)])
                            nc.sync.dma_start(
                                dst, asb[h2 * 4:(h2 + 1) * 4, :, :, :]
                                .rearrange("p a b d -> p (a b) d"))
                        at = mp.tile([128, NC], BF16, tag=f"aggT{g}")
                        src = bass.AP(agg_d, ((ch * 2 + g) * NC) * 128,
                                      [(128, NC), (1, 128)])
                        nc.sync.dma_start_transpose(at[:], src)
                        aggT[g] = at

                    # ---- out projection ----
                    osb = mp.tile([128, 2, NC], BF16, tag="osb")
                    for coh in range(2):
                        pout = pup.tile([128, NC], F32, tag="pout")
                        for g in range(2):
                            nc.tensor.matmul(
                                pout[:],
                                wout_t[:, g, coh * 128:(coh + 1) * 128],
                                aggT[g][:], start=(g == 0), stop=(g == 1))
                        nc.scalar.activation(osb[:, coh, :], pout[:],
                                             ACT.Identity,
                                             bias=bout_t[:, coh:coh + 1])
                    for coh in range(2):
                        for nh in range(2):
                            on = mp.tile([128, 128], BF16, tag="on")
                            nc.sync.dma_start_transpose(
                                on[:], osb[:, coh, nh * 128:(nh + 1) * 128])
                            dst = bass.AP(out16,
                                          (n0 + nh * 128) * 256 + coh * 128,
                                          [(256, 128), (1, 128)])
                            nc.sync.dma_start(dst, on[:])
    nc.compile()
    return nc


# ====================== cached SPMD runner ======================

class CachedSpmd:
    """run_bass_kernel_spmd equivalent with a persistent jitted callable
    (the stock helper re-traces shard_map on every call)."""

    def __init__(self, nc, n_cores=8):
        import jax
        from jax.experimental.shard_map import shard_map
        from jax.sharding import Mesh, PartitionSpec
        from concourse import bass2jax

        bass2jax.install_neuronx_cc_hook()
        self.n_cores = n_cores
        in_names, out_names, out_avals = [], [], []
        pname = nc.partition_id_tensor.name if nc.partition_id_tensor else None
        for alloc in nc.m.functions[0].allocations:
            if not isinstance(alloc, mybir.MemoryLocationSet):
                continue
            name = alloc.memorylocations[0].name
            if alloc.kind == "ExternalInput":
                if name != pname:
                    in_names.append(name)
            elif alloc.kind == "ExternalOutput":
                out_avals.append(jax.core.ShapedArray(
                    tuple(alloc.tensor_shape), mybir.dt.np(alloc.dtype)))
                out_names.append(name)
        self.in_names = in_names
        self.out_names = out_names
        self.out_avals = out_avals
        all_in = in_names + out_names
        if pname is not None:
            all_in = all_in + [pname]
        donate = tuple(range(len(in_names), len(in_names) + len(out_names)))

        def _body(*args):
            operands = list(args)
            if pname is not None:
                operands.append(bass2jax.partition_id_tensor())
            return tuple(bass2jax._bass_exec_p.bind(
                *operands,
                out_avals=tuple(out_avals),
                in_names=tuple(all_in),
                out_names=tuple(out_names),
                lowering_input_output_aliases=(),
                sim_require_finite=True,
                sim_require_nnan=True,
                nc=nc,
            ))

        devices = jax.devices()[:n_cores]
        mesh = Mesh(np.asarray(devices), ("core",))
        nin = len(in_names) + len(out_names)
        self._fn = jax.jit(
            shard_map(_body, mesh=mesh,
                      in_specs=(PartitionSpec("core"),) * nin,
                      out_specs=(PartitionSpec("core"),) * len(out_names),
                      check_rep=False),
            donate_argnums=donate, keep_unused=True)

    def __call__(self, in_maps):
        n = self.n_cores
        concat_in = [
            np.concatenate([np.asarray(m[name]) for m in in_maps], axis=0)
            for name in self.in_names
        ]
        concat_zeros = [
            np.zeros((n * a.shape[0], *a.shape[1:]), a.dtype)
            for a in self.out_avals
        ]
        out = self._fn(*concat_in, *concat_zeros)
        return [
            {name: np.asarray(out[i]).reshape(n, *self.out_avals[i].shape)[c]
             for i, name in enumerate(self.out_names)}
            for c in range(n)
        ]


# ====================== host-side packing ======================

def pack_weights(W_off, b_off, W_attn, b_attn, W_out, b_out):
    Wo = np.asarray(W_off, np.float32).reshape(C, 32, 2)
    wq = np.ascontiguousarray(np.concatenate(
        [Wo[:, :, 0], Wo[:, :, 1], np.asarray(W_attn, np.float32)], axis=1))
    bo = np.asarray(b_off, np.float32).reshape(32, 2)
    boa = np.ascontiguousarray(np.concatenate(
        [bo[:, 0], bo[:, 1], np.asarray(b_attn, np.float32)])[:, None])
    wout = np.asarray(W_out, np.float32).astype(ml_dtypes.bfloat16)
    bout = np.ascontiguousarray(
        np.asarray(b_out, np.float32).reshape(2, 128).T)
    hb = (np.arange(32) // 4 % 4 * RPH).astype(np.float32)[:, None]
    patt = np.zeros((32, 8), np.float32)
    patt[np.arange(32), np.arange(32) // 4] = 1.0
    stat = np.zeros((128, 16), np.float32)
    for yr in range(2):
        for a in range(4):
            for lhp in range(16):
                stat[yr * 64 + a * 16 + lhp, (lhp // 4) * 4 + a] = 1.0
    id128 = np.eye(128, dtype=np.float32)
    return dict(wq=wq, boa=boa, wout=wout, bout=bout, hb=hb,
                patt=patt, stat16=stat, id128=id128)


# ====================== host fallback ======================

def _host_fallback(query, reference_points, value, W_off, b_off, W_attn,
                   b_attn, W_out, b_out):
    out = np.empty((B, N, C), np.float32)
    w_oa = np.concatenate([W_off, W_attn], axis=1).astype(np.float32)
    b_oa = np.concatenate([b_off, b_attn]).astype(np.float32)

    def one(b):
        oa = query[b].reshape(-1, C) @ w_oa + b_oa
        offs = oa[:, :64].reshape(N, Hh, P, 2)
        logits = oa[:, 64:96].reshape(N, Hh, P)
        ee = np.exp(logits - logits.max(axis=-1, keepdims=True))
        attn = ee / ee.sum(axis=-1, keepdims=True)
        ref = reference_points[b] * 2.0 - 1.0
        xx = (ref[:, None, None, 0] + offs[..., 0] + 1.0) * 64.0 - 0.5
        yy = (ref[:, None, None, 1] + offs[..., 1] + 1.0) * 64.0 - 0.5
        x0 = np.floor(xx).astype(np.int64)
        y0 = np.floor(yy).astype(np.int64)
        wx = (xx - x0).astype(np.float32)
        wy = (yy - y0).astype(np.float32)
        val = np.ascontiguousarray(
            value[b].reshape(Hh, D, HH, WW).transpose(0, 2, 3, 1))
        valf = val.reshape(Hh * HH * WW, D)
        hbase = (np.arange(Hh) * (HH * WW))[None, :, None]
        agg = np.zeros((N, Hh, D), np.float32)
        for dy, dx, w in ((0, 0, (1 - wx) * (1 - wy)), (0, 1, wx * (1 - wy)),
                          (1, 0, (1 - wx) * wy), (1, 1, wx * wy)):
            ix = x0 + dx
            iy = y0 + dy
            ok = (ix >= 0) & (ix < WW) & (iy >= 0) & (iy < HH)
            idx = hbase + np.clip(iy, 0, HH - 1) * WW + np.clip(ix, 0, WW - 1)
            gth = valf[idx]
            cw = (w * ok * attn).astype(np.float32)
            agg += np.matmul(cw.reshape(N * Hh, 1, P),
                             gth.reshape(N * Hh, P, D)).reshape(N, Hh, D)
        out[b] = agg.reshape(N, C) @ W_out + b_out

    with ThreadPoolExecutor(max_workers=B) as ex:
        list(ex.map(one, range(B)))
    return out


# ====================== entry point ======================

def kernel(query, reference_points, value, W_off, b_off, W_attn, b_attn,
           W_out, b_out, H=None, W=None):
    query = np.asarray(query, np.float32)
    reference_points = np.asarray(reference_points, np.float32)
    value = np.asarray(value, np.float32)
    W_off = np.asarray(W_off, np.float32)
    b_off = np.asarray(b_off, np.float32)
    W_attn = np.asarray(W_attn, np.float32)
    b_attn = np.asarray(b_attn, np.float32)
    W_out = np.asarray(W_out, np.float32)
    b_out = np.asarray(b_out, np.float32)

    try:
        if "runner" not in _CACHE:
            _CACHE["runner"] = CachedSpmd(build_nc(N), n_cores=B)
        packed = pack_weights(W_off, b_off, W_attn, b_attn, W_out, b_out)

        def prep(b):
            return dict(
                q16=query[b].astype(np.float16),
                v16=value[b].astype(ml_dtypes.bfloat16),
                rp=np.ascontiguousarray(reference_points[b]),
                **packed,
            )

        with ThreadPoolExecutor(max_workers=B) as ex:
            in_maps = list(ex.map(prep, range(B)))
        res = _CACHE["runner"](in_maps)
        out = np.stack([res[b]["out16"].astype(np.float32)
                        for b in range(B)], axis=0)
        if not np.isfinite(out).all():
            raise FloatingPointError("non-finite device output")
        return out
    except Exception:
        import traceback
        traceback.print_exc()
        return _host_fallback(query, reference_points, value, W_off, b_off,
                              W_attn, b_attn, W_out, b_out)


if __name__ == "__main__":
    build_nc(N)
    print("built ok")
